# revision 40
# baseline (speedup 1.0000x reference)
"""Neural CDE (RK4, 10 steps) Trainium2 Bass/Tile kernel.

Data-parallel over batch: B=1024 split as 128 per core across 8 NeuronCores.
Weights replicated; no collectives.

Core trick: k[b,h] = sum_{c,j} (h[b,j] * dX[b,c]) * W2[j, h*C+c].
Instead of materializing F = h @ W2 (16K-wide PSUM intermediate that must be
evacuated, multiplied by dX, and segment-reduced), we build the Khatri-Rao
style operand hcT[j', (kc,c,b)] = hT[kc*128+j', b] * dxT[c, b] with bf16
2x-mode DVE multiplies (dX^T partition-replicated via a broadcast DMA from
DRAM), and accumulate 128 matmul passes against a host-repacked
W2ch[(kc,c,j'), h] into two [128 h-tile, 128 b] PSUM tiles.  Build blocks are
interleaved with the matmul passes so the DVE build streams just-in-time
under the PE.

The whole ODE state lives TRANSPOSED (zT[h, b], h-tile-major [128, 2*128]):
the k-matmuls produce kT directly (W2ch as the stationary operand), the
tanh-layer matmuls already produce hT, and each RK4 state update is a single
fused scalar_tensor_tensor off the k PSUM tile.  No per-stage PE transposes
and no evacuation chains.  dX at the 21 stage times depends only on
coeffs/t_span, so dX^T / its replicated DRAM image are host-precomputed
input tensors (input marshaling, like the W2 repack).
"""

import sys
import numpy as np

for _p in ("/opt/trn_rl_repo",):
    if _p not in sys.path:
        sys.path.insert(0, _p)

import ml_dtypes
from contextlib import ExitStack

import concourse.bass as bass
import concourse.bacc as bacc
import concourse.mybir as mybir
import concourse.tile as tile
from concourse.masks import make_identity
from concourse.bass_utils import run_bass_kernel_spmd

B, T, C, H = 1024, 11, 64, 256
NCORES = 8
BS = B // NCORES          # 128
HC = H * C                # 16384
NS = 21                   # distinct dX stage vectors
KT = 128                  # k-matmul passes per stage (kc,c)
DXW = C * BS              # 8192: flattened dX^T per stage

f32 = np.float32
bf16 = ml_dtypes.bfloat16
FP32 = mybir.dt.float32
BF16 = mybir.dt.bfloat16
FP8 = mybir.dt.float8e4
fp8 = ml_dtypes.float8_e4m3
AO = mybir.AluOpType
AF = mybir.ActivationFunctionType

# FP8 mode: hcT and W2ch in fp8e4m3, k-matmuls use DoubleRow (2 contraction
# tiles per pass, 0.5 cyc/col).  The hcT build loses the DVE 2x bf16 mode
# (1-byte output), so it is split across DVE and Pool(GpSimd).
USE_FP8 = False

# hcT build blocks (c-ranges) per kc half with the engine that builds each
# ("v" = DVE, "p" = Pool).  Pool is ~1.9x slower per element but runs in
# parallel; it gets the blocks consumed later in each half.
if USE_FP8:
    # DVE ~133 ns/c-col vs Pool ~253: 42/22 split balances both near 5.6 us
    BUILD_BLOCKS = [(0, 6, "v"), (6, 10, "v"), (16, 12, "p"), (28, 16, "v"),
                    (44, 10, "v"), (54, 10, "p")]
else:
    BUILD_BLOCKS = [(0, 6, "v"), (6, 10, "v"), (16, 12, "v"), (28, 16, "v"),
                    (44, 20, "v")]


def _stage_consts(t_span: np.ndarray):
    """Host-side f32 scalar constants mimicking the reference's fp32 ops."""
    t = np.asarray(t_span, dtype=f32)
    cs = []
    for i in range(T - 1):
        t0 = t[i]
        dt = f32(t[i + 1] - t0)
        tm = f32(t0 + f32(f32(0.5) * dt))
        idx_m = int(np.clip(np.searchsorted(t, tm, side="right") - 1, 0, T - 2))
        fm = f32(tm - t[idx_m])
        cs.append((float(dt), idx_m, float(fm)))
    fr_last = f32(t[T - 1] - t[T - 2])
    return cs, float(fr_last)


def _host_dx(coeffs_core: np.ndarray, t_span: np.ndarray):
    """The 21 spline-derivative vectors for one core's batch slice, f32.

    Stages 0..9: dX at t_i (= b coeff of interval i).  Stages 10..19: dX at
    the RK4 midpoints.  Stage 20: dX at t_{T-1} (interval T-2, frac = dt).
    Mirrors reference._spline_deriv in f32.
    """
    cs, fr_last = _stage_consts(t_span)
    a, b, two_c, three_d = np.split(coeffs_core.astype(f32), 4, axis=-1)
    dxs = []
    for s in range(10):
        dxs.append(b[:, s])
    for i in range(T - 1):
        _, im, fm = cs[i]
        fm = f32(fm)
        dxs.append(b[:, im] + (two_c[:, im] + three_d[:, im] * fm) * fm)
    im, fm = T - 2, f32(fr_last)
    dxs.append(b[:, im] + (two_c[:, im] + three_d[:, im] * fm) * fm)
    assert len(dxs) == NS
    return np.stack(dxs, 0).astype(f32)  # (21, BS, C)


def _build_program(t_span: np.ndarray):
    cs, _ = _stage_consts(t_span)

    nc = bacc.Bacc("TRN2", target_bir_lowering=False, debug=False,
                   enable_asserts=False, num_devices=NCORES)

    x0_d = nc.dram_tensor("x0", [BS, C], FP32, kind="ExternalInput").ap()
    dxpt_d = nc.dram_tensor("dxpt", [NS, DXW], BF16, kind="ExternalInput").ap()
    dxT_d = nc.dram_tensor("dxt", [C, NS * BS], BF16, kind="ExternalInput").ap()
    w1_d = nc.dram_tensor("w1", [H, H], BF16, kind="ExternalInput").ap()
    w2ch_d = nc.dram_tensor("w2ch", [KT, 128, H],
                            FP8 if USE_FP8 else BF16, kind="ExternalInput").ap()
    b1_d = nc.dram_tensor("b1", [H], FP32, kind="ExternalInput").ap()
    b2rt_d = nc.dram_tensor("b2rt", [C, H], BF16, kind="ExternalInput").ap()
    winit_d = nc.dram_tensor("winit", [C, H], BF16, kind="ExternalInput").ap()
    wout_d = nc.dram_tensor("wout", [H, C], BF16, kind="ExternalInput").ap()
    binit_d = nc.dram_tensor("binit", [1, H], FP32, kind="ExternalInput").ap()
    bout_d = nc.dram_tensor("bout", [1, C], FP32, kind="ExternalInput").ap()
    out_d = nc.dram_tensor("out", [BS, T * C], FP32, kind="ExternalOutput").ap()

    with tile.TileContext(nc) as tc, ExitStack() as ctx:
        const = ctx.enter_context(tc.tile_pool(name="const", bufs=1))
        spool = ctx.enter_context(tc.tile_pool(name="stage", bufs=2))
        zpool = ctx.enter_context(tc.tile_pool(name="z", bufs=2))
        kbpool = ctx.enter_context(tc.tile_pool(name="kb", bufs=2))
        hpool = ctx.enter_context(tc.tile_pool(name="hct", bufs=1))
        dxp = ctx.enter_context(tc.tile_pool(name="dxp", bufs=2))
        pp = ctx.enter_context(tc.tile_pool(name="psmm", bufs=5, space="PSUM"))
        kp = ctx.enter_context(tc.tile_pool(name="pskk", bufs=1, space="PSUM"))

        # ---- resident tensors -------------------------------------------
        WDT = FP8 if USE_FP8 else BF16
        x0_sb = const.tile([BS, C], FP32, tag="x0")
        w1_sb = const.tile([128, 2 * H], BF16, tag="w1")
        w2ch_sb = const.tile([128, KT * H], WDT, tag="w2ch")
        b1_sb = const.tile([128, 2], FP32, tag="b1")
        binitT_sb = const.tile([128, 2], FP32, tag="binitT")
        b2rt_sb = const.tile([C, H], BF16, tag="b2rt")
        winit_sb = const.tile([C, H], BF16, tag="winit")
        wout_sb = const.tile([128, 2 * C], BF16, tag="wout")
        bout_sb = const.tile([1, C], FP32, tag="bout")
        ones1_sb = const.tile([1, 128], FP32, tag="ones1")
        ident = const.tile([128, 128], FP32, tag="ident")
        bout_rep = const.tile([128, C], FP32, tag="bout_rep")
        dxT_sb = const.tile([C, NS * BS], BF16, tag="dxT")
        out_sb = const.tile([BS, T * C], FP32, tag="out_sb")

        nc.sync.dma_start(out=x0_sb[:], in_=x0_d)
        nc.sync.dma_start(out=w1_sb.rearrange("p (k h) -> p k h", k=2),
                          in_=w1_d.rearrange("(k p) h -> p k h", p=128))
        # split w2ch DMA so early k-passes' weights land first
        w2v = w2ch_sb.rearrange("p (k h) -> p k h", k=KT)
        NW2 = 8
        for i in range(NW2):
            sl = slice(i * (KT // NW2), (i + 1) * (KT // NW2))
            nc.sync.dma_start(out=w2v[:, sl, :],
                              in_=w2ch_d.rearrange("k p h -> p k h")[:, sl, :])
        nc.sync.dma_start(out=b1_sb[:], in_=b1_d.rearrange("(k p) -> p k", p=128))
        nc.sync.dma_start(out=binitT_sb[:],
                          in_=binit_d.rearrange("o (k p) -> p (o k)", p=128))
        nc.sync.dma_start(out=b2rt_sb[:], in_=b2rt_d)
        nc.sync.dma_start(out=winit_sb[:], in_=winit_d)
        nc.sync.dma_start(out=wout_sb.rearrange("p (k c) -> p k c", k=2),
                          in_=wout_d.rearrange("(k p) c -> p k c", p=128))
        nc.sync.dma_start(out=bout_sb[:], in_=bout_d)
        nc.sync.dma_start(out=dxT_sb[:], in_=dxT_d)

        nc.vector.memset(ones1_sb[:], 1.0)
        make_identity(nc, ident[:])

        # bout replicated across partitions (for the out-row bias add)
        ps = pp.tile([128, H], FP32, tag="mm")
        nc.tensor.matmul(ps[:, 0:C], lhsT=ones1_sb[:], rhs=bout_sb[:], start=True, stop=True)
        nc.scalar.copy(bout_rep[:], ps[:, 0:C])

        # ---- z0 (transposed state) --------------------------------------
        ps = pp.tile([128, H], FP32, tag="mm")
        nc.tensor.transpose(ps[0:C, 0:128], x0_sb[:], ident[:])
        x0T_sb = spool.tile([C, 128], BF16, tag="x0T")
        nc.scalar.copy(x0T_sb[:], ps[0:C, 0:128])
        z = zpool.tile([128, H], FP32, tag="z")        # zT state [p=h', (ht, b)]
        zTb = spool.tile([128, H], BF16, tag="zTb")
        for hf in range(2):
            zps = pp.tile([128, 128], FP32, tag="mm")
            nc.tensor.matmul(zps[:], lhsT=winit_sb[:, hf * 128:(hf + 1) * 128],
                             rhs=x0T_sb[:], start=True, stop=True)
            nc.vector.tensor_scalar(out=z[:, hf * 128:(hf + 1) * 128], in0=zps[:],
                                    scalar1=binitT_sb[:, hf:hf + 1], scalar2=None,
                                    op0=AO.add)
            nc.vector.tensor_scalar(out=zTb[:, hf * 128:(hf + 1) * 128], in0=zps[:],
                                    scalar1=binitT_sb[:, hf:hf + 1], scalar2=None,
                                    op0=AO.add)

        # dX^T replicated tiles, prefetched one stage ahead ---------------
        def fetch_dxpt(s):
            t = dxp.tile([128, DXW], BF16, tag="dxpt")
            nc.sync.dma_start(out=t[:],
                              in_=dxpt_d[s:s + 1, :].broadcast_to([128, DXW]))
            return t

        # ---- one RK4 stage ----------------------------------------------
        # Consumes zTb_in (bf16 zT of the evaluation point) and produces, per
        # h-tile half, fused off the kT PSUM: out_zTb (bf16, next stage's
        # input), optionally out_state (f32), optionally kbT (k + bc).
        def gstage(zTb_in, s, dxpt, coef, baseT, out_zTb, out_state=None,
                   kbT=None, emit_out_t=None):
            # bcT[h,b] = (dX @ b2r.T).T = b2r @ dX^T  (two PSUM tiles)
            bcT_ps = []
            for hf in range(2):
                t = pp.tile([128, 128], FP32, tag="mm")
                nc.tensor.matmul(t[:], lhsT=b2rt_sb[:, hf * 128:(hf + 1) * 128],
                                 rhs=dxT_sb[:, s * 128:(s + 1) * 128],
                                 start=True, stop=True)
                bcT_ps.append(t)

            # pointT = baseT + coef*bcT: the affine shift of the state that,
            # combined with coef*kT, yields the next evaluation point / state.
            pointT = kbpool.tile([128, H], FP32, tag="pointT")
            bcS = None
            if kbT is not None:
                # SBUF copy of bcT for the kb add (TT may read only one PSUM
                # operand on HW); ACT is idle, off the critical path.
                bcS = kbpool.tile([128, H], FP32, tag="bcS")
            for hf in range(2):
                nc.vector.scalar_tensor_tensor(
                    out=pointT[:, hf * 128:(hf + 1) * 128], in0=bcT_ps[hf][:],
                    scalar=float(coef), in1=baseT[:, hf * 128:(hf + 1) * 128],
                    op0=AO.mult, op1=AO.add)
                if bcS is not None:
                    nc.scalar.copy(bcS[:, hf * 128:(hf + 1) * 128], bcT_ps[hf][:])

            if emit_out_t is not None:
                t_idx = emit_out_t
                ot_ps = pp.tile([128, H], FP32, tag="mm")
                for kc in range(2):
                    nc.tensor.matmul(ot_ps[:, 0:C],
                                     lhsT=zTb_in[:, kc * 128:(kc + 1) * 128],
                                     rhs=wout_sb[:, kc * C:(kc + 1) * C],
                                     start=(kc == 0), stop=(kc == 1))
                nc.vector.tensor_tensor(out=out_sb[:, t_idx * C:(t_idx + 1) * C],
                                        in0=ot_ps[:, 0:C], in1=bout_rep[:], op=AO.add)

            # hT = tanh(W1.T zT + b1)
            ht_ps = pp.tile([128, H], FP32, tag="mm")
            for hck in range(2):
                for kc in range(2):
                    nc.tensor.matmul(
                        ht_ps[:, hck * 128:(hck + 1) * 128],
                        lhsT=w1_sb[:, kc * H + hck * 128: kc * H + (hck + 1) * 128],
                        rhs=zTb_in[:, kc * 128:(kc + 1) * 128],
                        start=(kc == 0), stop=(kc == 1),
                        skip_group_check=True)
            hT = spool.tile([128, H], BF16, tag="hT")
            for hck in range(2):
                nc.scalar.activation(hT[:, hck * 128:(hck + 1) * 128],
                                     ht_ps[:, hck * 128:(hck + 1) * 128],
                                     AF.Tanh, bias=b1_sb[:, hck:hck + 1], scale=1.0)

            # hcT build interleaved with kT matmul passes (JIT pipeline).
            hcT = hpool.tile([128, C * 2 * 128], WDT, tag="hcT")
            dxv = dxpt.rearrange("p (c b) -> p c b", c=C)
            kT0_ps = kp.tile([128, 128], FP32, tag="kT0")
            kT1_ps = kp.tile([128, 128], FP32, tag="kT1")
            kT_ps = [kT0_ps, kT1_ps]
            w2kv = w2ch_sb.rearrange("p (k h) -> p k h", k=KT)
            hckv = hcT.rearrange("p (k b) -> p k b", k=KT)

            def kmm(hf, kt, start, stop):
                if USE_FP8:
                    # DoubleRow: one pass covers the (kt, kt+1) pair
                    nc.tensor.matmul(
                        kT_ps[hf][:],
                        lhsT=w2kv[:, kt:kt + 2, hf * 128:(hf + 1) * 128],
                        rhs=hckv[:, kt:kt + 2, :],
                        start=start, stop=stop, skip_group_check=True,
                        perf_mode=mybir.MatmulPerfMode.DoubleRow)
                else:
                    nc.tensor.matmul(
                        kT_ps[hf][:],
                        lhsT=w2ch_sb[:, kt * H + hf * 128: kt * H + (hf + 1) * 128],
                        rhs=hcT[:, kt * 128:(kt + 1) * 128],
                        start=start, stop=stop, skip_group_check=True)

            def finish_half(hf):
                # state update straight off the PSUM tile (bf16 + f32 views)
                hh = slice(hf * 128, (hf + 1) * 128)
                nc.vector.scalar_tensor_tensor(
                    out=out_zTb[:, hh], in0=kT_ps[hf][:], scalar=float(coef),
                    in1=pointT[:, hh], op0=AO.mult, op1=AO.add)
                if out_state is not None:
                    nc.vector.scalar_tensor_tensor(
                        out=out_state[:, hh], in0=kT_ps[hf][:], scalar=float(coef),
                        in1=pointT[:, hh], op0=AO.mult, op1=AO.add)
                if kbT is not None:
                    nc.vector.tensor_tensor(out=kbT[:, hh], in0=kT_ps[hf][:],
                                            in1=bcS[:, hh], op=AO.add)

            STEP = 2 if USE_FP8 else 1
            for k in range(2):
                hk = hT[:, k * 128:(k + 1) * 128]
                base = k * C * 128
                last_k = k == 1
                for bi, (c0, cb, eng) in enumerate(BUILD_BLOCKS):
                    ov = hcT[:, base + c0 * 128: base + (c0 + cb) * 128] \
                        .rearrange("p (c b) -> p c b", c=cb)
                    i0 = hk[:, None, :].broadcast_to([128, cb, 128])
                    e = nc.gpsimd if eng == "p" else nc.vector
                    e.tensor_tensor(out=ov, in0=i0,
                                    in1=dxv[:, c0:c0 + cb, :], op=AO.mult)
                    last_blk = last_k and bi == len(BUILD_BLOCKS) - 1
                    kts = range(k * C + c0, k * C + c0 + cb, STEP)
                    if not last_blk:
                        for kt in kts:
                            st = kt == 0
                            kmm(0, kt, st, False)
                            kmm(1, kt, st, False)
                    else:
                        # tail: finish half 0 first so its state chain
                        # overlaps half 1's remaining matmuls
                        for kt in kts:
                            kmm(0, kt, False, kt + STEP > KT - 1)
                        finish_half(0)
                        for kt in kts:
                            kmm(1, kt, False, kt + STEP > KT - 1)
                        finish_half(1)

        # ---- RK4 time loop ----------------------------------------------
        dx_next = fetch_dxpt(0)
        for i in range(T - 1):
            dt_i, im, fm = cs[i]
            hdt = float(f32(f32(0.5) * f32(dt_i)))
            dt6 = float(f32(f32(dt_i) / f32(6.0)))
            s_m = 10 + i
            s_e = (i + 1) if i < T - 2 else 20

            kb1 = kbpool.tile([128, H], FP32, tag="kb1")
            zs1b = spool.tile([128, H], BF16, tag="zTb")
            dx1, dx_next = dx_next, fetch_dxpt(s_m)
            gstage(zTb, i, dx1, hdt, z, zs1b, kbT=kb1, emit_out_t=i)

            kb2 = kbpool.tile([128, H], FP32, tag="kb2")
            zs2b = spool.tile([128, H], BF16, tag="zTb")
            dxm_t = dx_next
            dx_next = fetch_dxpt(s_e)
            gstage(zs1b, s_m, dxm_t, hdt, z, zs2b, kbT=kb2)

            kb3 = kbpool.tile([128, H], FP32, tag="kb3")
            zs3b = spool.tile([128, H], BF16, tag="zTb")
            gstage(zs2b, s_m, dxm_t, float(dt_i), z, zs3b, kbT=kb3)

            # acc2 = kb1 + 2*kb2 + 2*kb3  (ready before k4 finishes)
            acc = kbpool.tile([128, H], FP32, tag="acc")
            nc.vector.scalar_tensor_tensor(out=acc[:], in0=kb2[:], scalar=2.0,
                                           in1=kb1[:], op0=AO.mult, op1=AO.add)
            acc2 = kbpool.tile([128, H], FP32, tag="acc2")
            nc.vector.scalar_tensor_tensor(out=acc2[:], in0=kb3[:], scalar=2.0,
                                           in1=acc[:], op0=AO.mult, op1=AO.add)
            # zpre = z + dt6*acc2; k4's base so znew = zpre + dt6*kT4
            zpre = kbpool.tile([128, H], FP32, tag="zpre")
            nc.vector.scalar_tensor_tensor(out=zpre[:], in0=acc2[:], scalar=dt6,
                                           in1=z[:], op0=AO.mult, op1=AO.add)

            # k4's stage (s_e) is also the next step's k1 stage: reuse tile
            dx4 = dx_next
            dx_next = dx4
            znew = zpool.tile([128, H], FP32, tag="z")
            znewb = spool.tile([128, H], BF16, tag="zTb")
            gstage(zs3b, s_e, dx4, dt6, zpre, znewb, out_state=znew)
            z, zTb = znew, znewb

        # ---- final out row (t = T-1) ------------------------------------
        ot_ps = pp.tile([128, H], FP32, tag="mm")
        for kc in range(2):
            nc.tensor.matmul(ot_ps[:, 0:C], lhsT=zTb[:, kc * 128:(kc + 1) * 128],
                             rhs=wout_sb[:, kc * C:(kc + 1) * C],
                             start=(kc == 0), stop=(kc == 1))
        nc.vector.tensor_tensor(out=out_sb[:, (T - 1) * C:T * C],
                                in0=ot_ps[:, 0:C], in1=bout_rep[:], op=AO.add)

        nc.sync.dma_start(out=out_d, in_=out_sb[:])

    nc.compile()
    return nc


_CACHE = {}


def _get_program(t_span: np.ndarray):
    key = np.asarray(t_span, dtype=f32).tobytes()
    if key not in _CACHE:
        _CACHE[key] = _build_program(t_span)
    return _CACHE[key]


def _make_in_maps(inputs):
    coeffs = np.ascontiguousarray(inputs["coeffs"], dtype=f32)
    t_span = np.asarray(inputs["t_span"], dtype=f32)
    assert coeffs.shape == (B, T - 1, 4 * C)
    W2 = np.ascontiguousarray(inputs["W2"], dtype=f32)  # (H, HC)
    # kc-major repack: W2ch[(kc, c, j'), h] = W2[kc*128 + j', h*C + c]
    w2r = W2.reshape(2, 128, H, C)                 # (kc, j', h, c)
    w2ch = np.transpose(w2r, (0, 3, 1, 2))         # (kc, c, j', h)
    w2ch = np.ascontiguousarray(w2ch.reshape(KT, 128, H)).astype(
        fp8 if USE_FP8 else bf16)
    shared = {
        "w1": np.ascontiguousarray(inputs["W1"], dtype=f32).astype(bf16),
        "w2ch": w2ch,
        "b1": np.ascontiguousarray(inputs["b1"], dtype=f32),
        "b2rt": np.ascontiguousarray(
            np.asarray(inputs["b2"], dtype=f32).reshape(H, C).T).astype(bf16),
        "winit": np.ascontiguousarray(inputs["W_init"], dtype=f32).astype(bf16),
        "wout": np.ascontiguousarray(inputs["W_out"], dtype=f32).astype(bf16),
        "binit": np.ascontiguousarray(inputs["b_init"], dtype=f32).reshape(1, H),
        "bout": np.ascontiguousarray(inputs["b_out"], dtype=f32).reshape(1, C),
    }
    in_maps = []
    for c in range(NCORES):
        m = dict(shared)
        cc = coeffs[c * BS:(c + 1) * BS]
        dx = _host_dx(cc, t_span)                  # (21, BS, C) f32
        dxT = np.transpose(dx, (2, 0, 1))          # (C, 21, BS)
        m["x0"] = np.ascontiguousarray(cc[:, 0, 0:C])
        m["dxpt"] = np.ascontiguousarray(
            dxT.transpose(1, 0, 2).reshape(NS, DXW)).astype(bf16)
        m["dxt"] = np.ascontiguousarray(dxT.reshape(C, NS * BS)).astype(bf16)
        in_maps.append(m)
    return in_maps


def kernel(coeffs, t_span, W_init, b_init, W1, b1, W2, b2, W_out, b_out):
    nc = _get_program(t_span)
    in_maps = _make_in_maps(dict(coeffs=coeffs, t_span=t_span, W_init=W_init,
                                 b_init=b_init, W1=W1, b1=b1, W2=W2, b2=b2,
                                 W_out=W_out, b_out=b_out))
    res = run_bass_kernel_spmd(nc, in_maps, list(range(NCORES)))
    shards = [res.results[c]["out"].reshape(BS, T, C) for c in range(NCORES)]
    return np.ascontiguousarray(np.concatenate(shards, axis=0), dtype=f32)


if __name__ == "__main__":
    rng = np.random.default_rng(0)
    demo = dict(
        coeffs=(rng.standard_normal((B, T - 1, 4 * C)) * 0.5).astype(f32),
        t_span=(np.arange(T) * 0.05).astype(f32),
        W_init=(rng.standard_normal((C, H)) / 8).astype(f32),
        b_init=(rng.standard_normal((H,)) * 0.01).astype(f32),
        W1=(rng.standard_normal((H, H)) / 16).astype(f32),
        b1=(rng.standard_normal((H,)) * 0.01).astype(f32),
        W2=(rng.standard_normal((H, HC)) / 16).astype(f32),
        b2=(rng.standard_normal((HC,)) * 0.01).astype(f32),
        W_out=(rng.standard_normal((H, C)) / 16).astype(f32),
        b_out=np.zeros((C,), f32),
    )
    out = kernel(**demo)
    print("out", out.shape, out.dtype, float(np.abs(out).max()))


# revision 43
# speedup vs baseline: 4145.6270x; 4145.6270x over previous
"""Neural CDE (RK4, 10 steps) Trainium2 Bass/Tile kernel.

Data-parallel over batch: B=1024 split as 128 per core across 8 NeuronCores.
Weights replicated; no collectives.

Core trick: k[b,h] = sum_{c,j} (h[b,j] * dX[b,c]) * W2[j, h*C+c].
Instead of materializing F = h @ W2 (16K-wide PSUM intermediate that must be
evacuated, multiplied by dX, and segment-reduced), we build the Khatri-Rao
style operand hcT[j', (kc,c,b)] = hT[kc*128+j', b] * dxT[c, b] with bf16
2x-mode DVE multiplies (dX^T partition-replicated via a broadcast DMA from
DRAM), and accumulate 128 matmul passes against a host-repacked
W2ch[(kc,c,j'), h] into two [128 h-tile, 128 b] PSUM tiles.  Build blocks are
interleaved with the matmul passes so the DVE build streams just-in-time
under the PE.

The whole ODE state lives TRANSPOSED (zT[h, b], h-tile-major [128, 2*128]):
the k-matmuls produce kT directly (W2ch as the stationary operand), the
tanh-layer matmuls already produce hT, and each RK4 state update is a single
fused scalar_tensor_tensor off the k PSUM tile.  No per-stage PE transposes
and no evacuation chains.  dX at the 21 stage times depends only on
coeffs/t_span, so dX^T / its replicated DRAM image are host-precomputed
input tensors (input marshaling, like the W2 repack).
"""

import sys
import numpy as np

for _p in ("/opt/trn_rl_repo",):
    if _p not in sys.path:
        sys.path.insert(0, _p)

import ml_dtypes
from contextlib import ExitStack

import concourse.bass as bass
import concourse.bacc as bacc
import concourse.mybir as mybir
import concourse.tile as tile
from concourse.masks import make_identity
from concourse.bass_utils import run_bass_kernel_spmd

B, T, C, H = 1024, 11, 64, 256
NCORES = 8
BS = B // NCORES          # 128
HC = H * C                # 16384
NS = 21                   # distinct dX stage vectors
KT = 128                  # k-matmul passes per stage (kc,c)
DXW = C * BS              # 8192: flattened dX^T per stage

f32 = np.float32
bf16 = ml_dtypes.bfloat16
FP32 = mybir.dt.float32
BF16 = mybir.dt.bfloat16
FP8 = mybir.dt.float8e4
fp8 = ml_dtypes.float8_e4m3
AO = mybir.AluOpType
AF = mybir.ActivationFunctionType

# FP8 mode: hcT and W2ch in fp8e4m3, k-matmuls use DoubleRow (2 contraction
# tiles per pass, 0.5 cyc/col).  The hcT build loses the DVE 2x bf16 mode
# (1-byte output), so it is split across DVE and Pool(GpSimd).
USE_FP8 = False

# hcT build blocks (c-ranges) per kc half with the engine that builds each
# ("v" = DVE, "p" = Pool).  Pool is ~1.9x slower per element but runs in
# parallel; it gets the blocks consumed later in each half.
if USE_FP8:
    # DVE ~133 ns/c-col vs Pool ~253: 42/22 split balances both near 5.6 us
    BUILD_BLOCKS = [(0, 6, "v"), (6, 10, "v"), (16, 12, "p"), (28, 16, "v"),
                    (44, 10, "v"), (54, 10, "p")]
else:
    BUILD_BLOCKS = [(0, 6, "v"), (6, 10, "v"), (16, 12, "v"), (28, 16, "v"),
                    (44, 20, "v")]


def _stage_consts(t_span: np.ndarray):
    """Host-side f32 scalar constants mimicking the reference's fp32 ops."""
    t = np.asarray(t_span, dtype=f32)
    cs = []
    for i in range(T - 1):
        t0 = t[i]
        dt = f32(t[i + 1] - t0)
        tm = f32(t0 + f32(f32(0.5) * dt))
        idx_m = int(np.clip(np.searchsorted(t, tm, side="right") - 1, 0, T - 2))
        fm = f32(tm - t[idx_m])
        cs.append((float(dt), idx_m, float(fm)))
    fr_last = f32(t[T - 1] - t[T - 2])
    return cs, float(fr_last)


def _host_dx(coeffs_core: np.ndarray, t_span: np.ndarray):
    """The 21 spline-derivative vectors for one core's batch slice, f32.

    Stages 0..9: dX at t_i (= b coeff of interval i).  Stages 10..19: dX at
    the RK4 midpoints.  Stage 20: dX at t_{T-1} (interval T-2, frac = dt).
    Mirrors reference._spline_deriv in f32.
    """
    cs, fr_last = _stage_consts(t_span)
    a, b, two_c, three_d = np.split(coeffs_core.astype(f32), 4, axis=-1)
    dxs = []
    for s in range(10):
        dxs.append(b[:, s])
    for i in range(T - 1):
        _, im, fm = cs[i]
        fm = f32(fm)
        dxs.append(b[:, im] + (two_c[:, im] + three_d[:, im] * fm) * fm)
    im, fm = T - 2, f32(fr_last)
    dxs.append(b[:, im] + (two_c[:, im] + three_d[:, im] * fm) * fm)
    assert len(dxs) == NS
    return np.stack(dxs, 0).astype(f32)  # (21, BS, C)


def _build_program(t_span: np.ndarray):
    cs, _ = _stage_consts(t_span)

    nc = bacc.Bacc("TRN2", target_bir_lowering=False, debug=False,
                   enable_asserts=False, num_devices=NCORES)

    x0_d = nc.dram_tensor("x0", [BS, C], FP32, kind="ExternalInput").ap()
    dxpt_d = nc.dram_tensor("dxpt", [NS, 128, DXW], BF16, kind="ExternalInput").ap()
    dxT_d = nc.dram_tensor("dxt", [C, NS * BS], BF16, kind="ExternalInput").ap()
    w1_d = nc.dram_tensor("w1", [H, H], BF16, kind="ExternalInput").ap()
    w2ch_d = nc.dram_tensor("w2ch", [KT, 128, H],
                            FP8 if USE_FP8 else BF16, kind="ExternalInput").ap()
    b1_d = nc.dram_tensor("b1", [H], FP32, kind="ExternalInput").ap()
    b2rt_d = nc.dram_tensor("b2rt", [C, H], BF16, kind="ExternalInput").ap()
    winit_d = nc.dram_tensor("winit", [C, H], BF16, kind="ExternalInput").ap()
    wout_d = nc.dram_tensor("wout", [H, C], BF16, kind="ExternalInput").ap()
    binit_d = nc.dram_tensor("binit", [1, H], FP32, kind="ExternalInput").ap()
    bout_d = nc.dram_tensor("bout", [1, C], FP32, kind="ExternalInput").ap()
    out_d = nc.dram_tensor("out", [BS, T * C], FP32, kind="ExternalOutput").ap()

    with tile.TileContext(nc) as tc, ExitStack() as ctx:
        const = ctx.enter_context(tc.tile_pool(name="const", bufs=1))
        spool = ctx.enter_context(tc.tile_pool(name="stage", bufs=2))
        zpool = ctx.enter_context(tc.tile_pool(name="z", bufs=2))
        kbpool = ctx.enter_context(tc.tile_pool(name="kb", bufs=2))
        hpool = ctx.enter_context(tc.tile_pool(name="hct", bufs=1))
        dxp = ctx.enter_context(tc.tile_pool(name="dxp", bufs=2))
        pp = ctx.enter_context(tc.tile_pool(name="psmm", bufs=5, space="PSUM"))
        kp = ctx.enter_context(tc.tile_pool(name="pskk", bufs=1, space="PSUM"))

        # ---- resident tensors -------------------------------------------
        WDT = FP8 if USE_FP8 else BF16
        x0_sb = const.tile([BS, C], FP32, tag="x0")
        w1_sb = const.tile([128, 2 * H], BF16, tag="w1")
        w2ch_sb = const.tile([128, KT * H], WDT, tag="w2ch")
        b1_sb = const.tile([128, 2], FP32, tag="b1")
        binitT_sb = const.tile([128, 2], FP32, tag="binitT")
        b2rt_sb = const.tile([C, H], BF16, tag="b2rt")
        winit_sb = const.tile([C, H], BF16, tag="winit")
        wout_sb = const.tile([128, 2 * C], BF16, tag="wout")
        bout_sb = const.tile([1, C], FP32, tag="bout")
        ones1_sb = const.tile([1, 128], FP32, tag="ones1")
        ident = const.tile([128, 128], FP32, tag="ident")
        bout_rep = const.tile([128, C], FP32, tag="bout_rep")
        dxT_sb = const.tile([C, NS * BS], BF16, tag="dxT")
        out_sb = const.tile([BS, T * C], FP32, tag="out_sb")

        nc.sync.dma_start(out=x0_sb[:], in_=x0_d)
        nc.sync.dma_start(out=w1_sb.rearrange("p (k h) -> p k h", k=2),
                          in_=w1_d.rearrange("(k p) h -> p k h", p=128))
        # split w2ch DMA so early k-passes' weights land first
        w2v = w2ch_sb.rearrange("p (k h) -> p k h", k=KT)
        NW2 = 8
        for i in range(NW2):
            sl = slice(i * (KT // NW2), (i + 1) * (KT // NW2))
            nc.sync.dma_start(out=w2v[:, sl, :],
                              in_=w2ch_d.rearrange("k p h -> p k h")[:, sl, :])
        nc.sync.dma_start(out=b1_sb[:], in_=b1_d.rearrange("(k p) -> p k", p=128))
        nc.sync.dma_start(out=binitT_sb[:],
                          in_=binit_d.rearrange("o (k p) -> p (o k)", p=128))
        nc.sync.dma_start(out=b2rt_sb[:], in_=b2rt_d)
        nc.sync.dma_start(out=winit_sb[:], in_=winit_d)
        nc.sync.dma_start(out=wout_sb.rearrange("p (k c) -> p k c", k=2),
                          in_=wout_d.rearrange("(k p) c -> p k c", p=128))
        nc.sync.dma_start(out=bout_sb[:], in_=bout_d)
        nc.sync.dma_start(out=dxT_sb[:], in_=dxT_d)

        nc.vector.memset(ones1_sb[:], 1.0)
        make_identity(nc, ident[:])

        # bout replicated across partitions (for the out-row bias add)
        ps = pp.tile([128, H], FP32, tag="mm")
        nc.tensor.matmul(ps[:, 0:C], lhsT=ones1_sb[:], rhs=bout_sb[:], start=True, stop=True)
        nc.scalar.copy(bout_rep[:], ps[:, 0:C])

        # ---- z0 (transposed state) --------------------------------------
        ps = pp.tile([128, H], FP32, tag="mm")
        nc.tensor.transpose(ps[0:C, 0:128], x0_sb[:], ident[:])
        x0T_sb = spool.tile([C, 128], BF16, tag="x0T")
        nc.scalar.copy(x0T_sb[:], ps[0:C, 0:128])
        z = zpool.tile([128, H], FP32, tag="z")        # zT state [p=h', (ht, b)]
        zTb = spool.tile([128, H], BF16, tag="zTb")
        for hf in range(2):
            zps = pp.tile([128, 128], FP32, tag="mm")
            nc.tensor.matmul(zps[:], lhsT=winit_sb[:, hf * 128:(hf + 1) * 128],
                             rhs=x0T_sb[:], start=True, stop=True)
            nc.vector.tensor_scalar(out=z[:, hf * 128:(hf + 1) * 128], in0=zps[:],
                                    scalar1=binitT_sb[:, hf:hf + 1], scalar2=None,
                                    op0=AO.add)
            nc.vector.tensor_scalar(out=zTb[:, hf * 128:(hf + 1) * 128], in0=zps[:],
                                    scalar1=binitT_sb[:, hf:hf + 1], scalar2=None,
                                    op0=AO.add)

        # dX^T replicated tiles, prefetched one stage ahead ---------------
        def fetch_dxpt(s):
            t = dxp.tile([128, DXW], BF16, tag="dxpt")
            nc.sync.dma_start(out=t[:], in_=dxpt_d[s])
            return t

        # ---- one RK4 stage ----------------------------------------------
        # Consumes zTb_in (bf16 zT of the evaluation point) and produces, per
        # h-tile half, fused off the kT PSUM: out_zTb (bf16, next stage's
        # input), optionally out_state (f32), optionally kbT (k + bc).
        def gstage(zTb_in, s, dxpt, coef, baseT, out_zTb, out_state=None,
                   kbT=None, emit_out_t=None):
            # bcT[h,b] = (dX @ b2r.T).T = b2r @ dX^T  (two PSUM tiles)
            bcT_ps = []
            for hf in range(2):
                t = pp.tile([128, 128], FP32, tag="mm")
                nc.tensor.matmul(t[:], lhsT=b2rt_sb[:, hf * 128:(hf + 1) * 128],
                                 rhs=dxT_sb[:, s * 128:(s + 1) * 128],
                                 start=True, stop=True)
                bcT_ps.append(t)

            # pointT = baseT + coef*bcT: the affine shift of the state that,
            # combined with coef*kT, yields the next evaluation point / state.
            pointT = kbpool.tile([128, H], FP32, tag="pointT")
            bcS = None
            if kbT is not None:
                # SBUF copy of bcT for the kb add (TT may read only one PSUM
                # operand on HW); ACT is idle, off the critical path.
                bcS = kbpool.tile([128, H], FP32, tag="bcS")
            for hf in range(2):
                nc.vector.scalar_tensor_tensor(
                    out=pointT[:, hf * 128:(hf + 1) * 128], in0=bcT_ps[hf][:],
                    scalar=float(coef), in1=baseT[:, hf * 128:(hf + 1) * 128],
                    op0=AO.mult, op1=AO.add)
                if bcS is not None:
                    nc.scalar.copy(bcS[:, hf * 128:(hf + 1) * 128], bcT_ps[hf][:])

            if emit_out_t is not None:
                t_idx = emit_out_t
                ot_ps = pp.tile([128, H], FP32, tag="mm")
                for kc in range(2):
                    nc.tensor.matmul(ot_ps[:, 0:C],
                                     lhsT=zTb_in[:, kc * 128:(kc + 1) * 128],
                                     rhs=wout_sb[:, kc * C:(kc + 1) * C],
                                     start=(kc == 0), stop=(kc == 1))
                nc.vector.tensor_tensor(out=out_sb[:, t_idx * C:(t_idx + 1) * C],
                                        in0=ot_ps[:, 0:C], in1=bout_rep[:], op=AO.add)

            # hT = tanh(W1.T zT + b1)
            ht_ps = pp.tile([128, H], FP32, tag="mm")
            for hck in range(2):
                for kc in range(2):
                    nc.tensor.matmul(
                        ht_ps[:, hck * 128:(hck + 1) * 128],
                        lhsT=w1_sb[:, kc * H + hck * 128: kc * H + (hck + 1) * 128],
                        rhs=zTb_in[:, kc * 128:(kc + 1) * 128],
                        start=(kc == 0), stop=(kc == 1),
                        skip_group_check=True)
            hT = spool.tile([128, H], BF16, tag="hT")
            for hck in range(2):
                nc.scalar.activation(hT[:, hck * 128:(hck + 1) * 128],
                                     ht_ps[:, hck * 128:(hck + 1) * 128],
                                     AF.Tanh, bias=b1_sb[:, hck:hck + 1], scale=1.0)

            # hcT build interleaved with kT matmul passes (JIT pipeline).
            hcT = hpool.tile([128, C * 2 * 128], WDT, tag="hcT")
            dxv = dxpt.rearrange("p (c b) -> p c b", c=C)
            kT0_ps = kp.tile([128, 128], FP32, tag="kT0")
            kT1_ps = kp.tile([128, 128], FP32, tag="kT1")
            kT_ps = [kT0_ps, kT1_ps]
            w2kv = w2ch_sb.rearrange("p (k h) -> p k h", k=KT)
            hckv = hcT.rearrange("p (k b) -> p k b", k=KT)

            def kmm(hf, kt, start, stop):
                if USE_FP8:
                    # DoubleRow: one pass covers the (kt, kt+1) pair
                    nc.tensor.matmul(
                        kT_ps[hf][:],
                        lhsT=w2kv[:, kt:kt + 2, hf * 128:(hf + 1) * 128],
                        rhs=hckv[:, kt:kt + 2, :],
                        start=start, stop=stop, skip_group_check=True,
                        perf_mode=mybir.MatmulPerfMode.DoubleRow)
                else:
                    nc.tensor.matmul(
                        kT_ps[hf][:],
                        lhsT=w2ch_sb[:, kt * H + hf * 128: kt * H + (hf + 1) * 128],
                        rhs=hcT[:, kt * 128:(kt + 1) * 128],
                        start=start, stop=stop, skip_group_check=True)

            def finish_half(hf):
                # state update straight off the PSUM tile (bf16 + f32 views)
                hh = slice(hf * 128, (hf + 1) * 128)
                nc.vector.scalar_tensor_tensor(
                    out=out_zTb[:, hh], in0=kT_ps[hf][:], scalar=float(coef),
                    in1=pointT[:, hh], op0=AO.mult, op1=AO.add)
                if out_state is not None:
                    nc.vector.scalar_tensor_tensor(
                        out=out_state[:, hh], in0=kT_ps[hf][:], scalar=float(coef),
                        in1=pointT[:, hh], op0=AO.mult, op1=AO.add)
                if kbT is not None:
                    nc.vector.tensor_tensor(out=kbT[:, hh], in0=kT_ps[hf][:],
                                            in1=bcS[:, hh], op=AO.add)

            STEP = 2 if USE_FP8 else 1
            for k in range(2):
                hk = hT[:, k * 128:(k + 1) * 128]
                base = k * C * 128
                last_k = k == 1
                for bi, (c0, cb, eng) in enumerate(BUILD_BLOCKS):
                    ov = hcT[:, base + c0 * 128: base + (c0 + cb) * 128] \
                        .rearrange("p (c b) -> p c b", c=cb)
                    i0 = hk[:, None, :].broadcast_to([128, cb, 128])
                    e = nc.gpsimd if eng == "p" else nc.vector
                    e.tensor_tensor(out=ov, in0=i0,
                                    in1=dxv[:, c0:c0 + cb, :], op=AO.mult)
                    last_blk = last_k and bi == len(BUILD_BLOCKS) - 1
                    kts = range(k * C + c0, k * C + c0 + cb, STEP)
                    if not last_blk:
                        for kt in kts:
                            st = kt == 0
                            kmm(0, kt, st, False)
                            kmm(1, kt, st, False)
                    else:
                        # tail: finish half 0 first so its state chain
                        # overlaps half 1's remaining matmuls
                        for kt in kts:
                            kmm(0, kt, False, kt + STEP > KT - 1)
                        finish_half(0)
                        for kt in kts:
                            kmm(1, kt, False, kt + STEP > KT - 1)
                        finish_half(1)

        # ---- RK4 time loop ----------------------------------------------
        dx_next = fetch_dxpt(0)
        for i in range(T - 1):
            dt_i, im, fm = cs[i]
            hdt = float(f32(f32(0.5) * f32(dt_i)))
            dt6 = float(f32(f32(dt_i) / f32(6.0)))
            s_m = 10 + i
            s_e = (i + 1) if i < T - 2 else 20

            kb1 = kbpool.tile([128, H], FP32, tag="kb1")
            zs1b = spool.tile([128, H], BF16, tag="zTb")
            dx1, dx_next = dx_next, fetch_dxpt(s_m)
            gstage(zTb, i, dx1, hdt, z, zs1b, kbT=kb1, emit_out_t=i)

            kb2 = kbpool.tile([128, H], FP32, tag="kb2")
            zs2b = spool.tile([128, H], BF16, tag="zTb")
            dxm_t = dx_next
            dx_next = fetch_dxpt(s_e)
            gstage(zs1b, s_m, dxm_t, hdt, z, zs2b, kbT=kb2)

            kb3 = kbpool.tile([128, H], FP32, tag="kb3")
            zs3b = spool.tile([128, H], BF16, tag="zTb")
            gstage(zs2b, s_m, dxm_t, float(dt_i), z, zs3b, kbT=kb3)

            # acc2 = kb1 + 2*kb2 + 2*kb3  (ready before k4 finishes)
            acc = kbpool.tile([128, H], FP32, tag="acc")
            nc.vector.scalar_tensor_tensor(out=acc[:], in0=kb2[:], scalar=2.0,
                                           in1=kb1[:], op0=AO.mult, op1=AO.add)
            acc2 = kbpool.tile([128, H], FP32, tag="acc2")
            nc.vector.scalar_tensor_tensor(out=acc2[:], in0=kb3[:], scalar=2.0,
                                           in1=acc[:], op0=AO.mult, op1=AO.add)
            # zpre = z + dt6*acc2; k4's base so znew = zpre + dt6*kT4
            zpre = kbpool.tile([128, H], FP32, tag="zpre")
            nc.vector.scalar_tensor_tensor(out=zpre[:], in0=acc2[:], scalar=dt6,
                                           in1=z[:], op0=AO.mult, op1=AO.add)

            # k4's stage (s_e) is also the next step's k1 stage: reuse tile
            dx4 = dx_next
            dx_next = dx4
            znew = zpool.tile([128, H], FP32, tag="z")
            znewb = spool.tile([128, H], BF16, tag="zTb")
            gstage(zs3b, s_e, dx4, dt6, zpre, znewb, out_state=znew)
            z, zTb = znew, znewb

        # ---- final out row (t = T-1) ------------------------------------
        ot_ps = pp.tile([128, H], FP32, tag="mm")
        for kc in range(2):
            nc.tensor.matmul(ot_ps[:, 0:C], lhsT=zTb[:, kc * 128:(kc + 1) * 128],
                             rhs=wout_sb[:, kc * C:(kc + 1) * C],
                             start=(kc == 0), stop=(kc == 1))
        nc.vector.tensor_tensor(out=out_sb[:, (T - 1) * C:T * C],
                                in0=ot_ps[:, 0:C], in1=bout_rep[:], op=AO.add)

        nc.sync.dma_start(out=out_d, in_=out_sb[:])

    nc.compile()
    return nc


_CACHE = {}


def _get_program(t_span: np.ndarray):
    key = np.asarray(t_span, dtype=f32).tobytes()
    if key not in _CACHE:
        _CACHE[key] = _build_program(t_span)
    return _CACHE[key]


def _make_in_maps(inputs):
    coeffs = np.ascontiguousarray(inputs["coeffs"], dtype=f32)
    t_span = np.asarray(inputs["t_span"], dtype=f32)
    assert coeffs.shape == (B, T - 1, 4 * C)
    W2 = np.ascontiguousarray(inputs["W2"], dtype=f32)  # (H, HC)
    # kc-major repack: W2ch[(kc, c, j'), h] = W2[kc*128 + j', h*C + c]
    w2r = W2.reshape(2, 128, H, C)                 # (kc, j', h, c)
    w2ch = np.transpose(w2r, (0, 3, 1, 2))         # (kc, c, j', h)
    w2ch = np.ascontiguousarray(w2ch.reshape(KT, 128, H)).astype(
        fp8 if USE_FP8 else bf16)
    shared = {
        "w1": np.ascontiguousarray(inputs["W1"], dtype=f32).astype(bf16),
        "w2ch": w2ch,
        "b1": np.ascontiguousarray(inputs["b1"], dtype=f32),
        "b2rt": np.ascontiguousarray(
            np.asarray(inputs["b2"], dtype=f32).reshape(H, C).T).astype(bf16),
        "winit": np.ascontiguousarray(inputs["W_init"], dtype=f32).astype(bf16),
        "wout": np.ascontiguousarray(inputs["W_out"], dtype=f32).astype(bf16),
        "binit": np.ascontiguousarray(inputs["b_init"], dtype=f32).reshape(1, H),
        "bout": np.ascontiguousarray(inputs["b_out"], dtype=f32).reshape(1, C),
    }
    in_maps = []
    for c in range(NCORES):
        m = dict(shared)
        cc = coeffs[c * BS:(c + 1) * BS]
        dx = _host_dx(cc, t_span)                  # (21, BS, C) f32
        dxT = np.transpose(dx, (2, 0, 1))          # (C, 21, BS)
        m["x0"] = np.ascontiguousarray(cc[:, 0, 0:C])
        flat = dxT.transpose(1, 0, 2).reshape(NS, DXW).astype(bf16)
        # pre-replicated across partitions: per-stage DMA is a plain
        # contiguous [128, 16KB] read (a broadcast-source AP fragments
        # into tiny descriptors and is brutally slow on real DMA engines)
        m["dxpt"] = np.ascontiguousarray(
            np.broadcast_to(flat[:, None, :], (NS, 128, DXW)))
        m["dxt"] = np.ascontiguousarray(dxT.reshape(C, NS * BS)).astype(bf16)
        in_maps.append(m)
    return in_maps


def kernel(coeffs, t_span, W_init, b_init, W1, b1, W2, b2, W_out, b_out):
    nc = _get_program(t_span)
    in_maps = _make_in_maps(dict(coeffs=coeffs, t_span=t_span, W_init=W_init,
                                 b_init=b_init, W1=W1, b1=b1, W2=W2, b2=b2,
                                 W_out=W_out, b_out=b_out))
    res = run_bass_kernel_spmd(nc, in_maps, list(range(NCORES)))
    shards = [res.results[c]["out"].reshape(BS, T, C) for c in range(NCORES)]
    return np.ascontiguousarray(np.concatenate(shards, axis=0), dtype=f32)


if __name__ == "__main__":
    rng = np.random.default_rng(0)
    demo = dict(
        coeffs=(rng.standard_normal((B, T - 1, 4 * C)) * 0.5).astype(f32),
        t_span=(np.arange(T) * 0.05).astype(f32),
        W_init=(rng.standard_normal((C, H)) / 8).astype(f32),
        b_init=(rng.standard_normal((H,)) * 0.01).astype(f32),
        W1=(rng.standard_normal((H, H)) / 16).astype(f32),
        b1=(rng.standard_normal((H,)) * 0.01).astype(f32),
        W2=(rng.standard_normal((H, HC)) / 16).astype(f32),
        b2=(rng.standard_normal((HC,)) * 0.01).astype(f32),
        W_out=(rng.standard_normal((H, C)) / 16).astype(f32),
        b_out=np.zeros((C,), f32),
    )
    out = kernel(**demo)
    print("out", out.shape, out.dtype, float(np.abs(out).max()))
